# revision 32
# baseline (speedup 1.0000x reference)
"""DiffAttn kernel for 8 trn2 NeuronCores (v2.4: bf16, proj/attention interleave).

Problem (per reference):
  X [4, 4096, 1024]; Wq/Wk [1024, 256]; Wv [1024, 128]; biases; lam scalar.
  Q,K = X@Wq+bq, X@Wk+bk ; V = X@Wv+bv
  A_i = Q_i @ K_i^T / sqrt(128)  (i = 1,2 : the two 128-wide halves)
  out = (softmax(A1) - lam * softmax(A2)) @ V          -> [4, 4096, 128]

Sharding: 8 cores = 4 batches x 2 query-halves. Each core computes the
attention output for 2048 queries of one batch; K/V projections for the
full 4096 keys of that batch are computed redundantly on both cores of
the pair. Host passes X^T per core (bf16), its query rows first.

v2.4 dataflow (all matmul operands bf16, PSUM fp32):
  ScalarE's exp stream is the pacer (~1.2us per 128x1024 step), so the
  PE's projection work is interleaved INTO the attention step stream as
  single-chunk jobs to keep the PE from idling (HAM downclocks an idle
  PE). Attention runs in 4 supers of 512 queries, split into 16-key-tile
  half-windows scheduled (s0,h0),(s1,h0),(s0,h1),(s1,h1),(s2,h0)... so
  the PV PSUM accumulators only span one window (2 banks instead of 4)
  with DVE merging partials in SBUF; freed banks hold the projection
  PSUM. Per step: 2 scores matmuls into a double-buffered [128, 2x512]
  fp32 PSUM tile, one FD=1024 exp ACTIVATE -> e12 bf16, DVE accumulates
  softmax denominators bf16 at 2x, 2 PV matmuls. V reaches its [key, d]
  layout via DMA-xbar transposes (no PE/PSUM). Finalize per super:
  ones-matmul broadcasts the denominator partition-sums into a borrowed
  scores buffer, DVE reciprocal_approx_fast + 3 tensor ops; lam folds
  into the reciprocal. Output ships as O^T [128, 2048] fp32.
"""

import sys

sys.path.insert(0, "/opt/trn_rl_repo")

import numpy as np

import concourse.bacc as bacc
import concourse.mybir as mybir
from concourse.tile import TileContext
from concourse.bass_utils import run_bass_kernel_spmd

F32 = mybir.dt.float32
BF16 = mybir.dt.bfloat16
AF = mybir.ActivationFunctionType

D = 128
EMB = 1024
B, S = 4, 4096
NQ = S // 2          # queries per core
SQC = 512            # projection column chunk
NCC = S // SQC       # 8 projection column chunks
NE = EMB // 128      # 8 emb k-tiles
SUP = 512            # attention query super-chunk
NSUP = NQ // SUP     # 4
NSK = S // 128       # 32 key tiles
HKT = 16             # key tiles per segment (half window)
INV_SQRT_D = 1.0 / np.sqrt(np.float32(D))

# segment schedule: supers in pairs, half-windows alternated so PV PSUM
# only ever holds one window and chunk c isn't needed before step 8*c
SEGS = [(0, 0), (1, 0), (0, 1), (1, 1), (2, 0), (3, 0), (2, 1), (3, 1)]

TRACE = False
TRACE_DIR = None
LAST_RESULT = None


def _build():
    nc = bacc.Bacc("TRN2", target_bir_lowering=False, debug=False, num_devices=8)

    xt = nc.dram_tensor("xt", [EMB, S], BF16, kind="ExternalInput")
    wq = nc.dram_tensor("wq", [EMB, 2 * D], BF16, kind="ExternalInput")
    wk = nc.dram_tensor("wk", [EMB, 2 * D], BF16, kind="ExternalInput")
    wv = nc.dram_tensor("wv", [EMB, D], BF16, kind="ExternalInput")
    bq = nc.dram_tensor("bq", [2 * D, 1], F32, kind="ExternalInput")
    bk = nc.dram_tensor("bk", [2 * D, 1], F32, kind="ExternalInput")
    bv = nc.dram_tensor("bv", [D, 1], F32, kind="ExternalInput")
    lamv = nc.dram_tensor("lamv", [128, 1], F32, kind="ExternalInput")
    out = nc.dram_tensor("o", [D, NQ], F32, kind="ExternalOutput")  # O^T
    # pair-AllGather scratch: two pieces of (k1|k2|v) x 1024 own keys
    kv_in = [nc.dram_tensor(f"kvi{p}", [128, 3 * 1024], BF16) for p in range(2)]
    kv_out = [nc.dram_tensor(f"kvo{p}", [2, 128, 3 * 1024], BF16) for p in range(2)]

    from contextlib import ExitStack

    with TileContext(nc) as tc, ExitStack() as ctx:
        wpool = ctx.enter_context(tc.tile_pool(name="w", bufs=1))
        wk1 = wpool.tile([128, NE, 128], BF16, tag="wk1")
        wk2 = wpool.tile([128, NE, 128], BF16, tag="wk2")
        wq1 = wpool.tile([128, NE, 128], BF16, tag="wq1")
        wq2 = wpool.tile([128, NE, 128], BF16, tag="wq2")
        wvt = wpool.tile([128, NE, 128], BF16, tag="wvt")

        xpool = ctx.enter_context(tc.tile_pool(name="xt", bufs=1))
        # only the own half (chunks 0-3) is ever projected locally
        xt_all = xpool.tile([128, NCC // 2, NE, SQC], BF16, tag="xall")

        def wsrc(w, dsl):
            return w[:, dsl].rearrange("(t p) d -> p t d", p=128)

        def load_chunk(cc):
            nc.sync.dma_start(
                out=xt_all[:, cc],
                in_=xt[:, cc * SQC : (cc + 1) * SQC].rearrange(
                    "(t p) s -> p t s", p=128
                ),
            )

        def load_chunk_split(cc):
            csl = slice(cc * SQC, (cc + 1) * SQC)
            for e in range(NE):
                nc.sync.dma_start(
                    out=xt_all[:, cc, e, :],
                    in_=xt[e * 128 : (e + 1) * 128, csl],
                )

        nc.sync.dma_start(out=wk1[:], in_=wsrc(wk, slice(0, 128)))
        load_chunk_split(0)
        nc.sync.dma_start(out=wk2[:], in_=wsrc(wk, slice(128, 256)))
        load_chunk_split(1)
        nc.sync.dma_start(out=wq1[:], in_=wsrc(wq, slice(0, 128)))
        nc.sync.dma_start(out=wq2[:], in_=wsrc(wq, slice(128, 256)))
        nc.sync.dma_start(out=wvt[:], in_=wsrc(wv, slice(0, 128)))
        for cc in range(2, NCC // 2):
            load_chunk(cc)

        cpool = ctx.enter_context(tc.tile_pool(name="const", bufs=1))
        ones_sq = cpool.tile([128, 128], BF16, tag="ones_sq")
        nc.vector.memset(ones_sq[:], 1.0)
        ident = cpool.tile([128, 128], BF16, tag="ident")
        from concourse import masks as _masks

        _masks.make_identity(nc, ident[:])

        bq1 = cpool.tile([128, 1], F32, tag="bq1")
        bq2 = cpool.tile([128, 1], F32, tag="bq2")
        bk1 = cpool.tile([128, 1], F32, tag="bk1")
        bk2 = cpool.tile([128, 1], F32, tag="bk2")
        bvt = cpool.tile([128, 1], F32, tag="bvt")
        lam_t = cpool.tile([128, 1], F32, tag="lam")
        nc.gpsimd.dma_start(out=bk1[:], in_=bk[0:128, :])
        nc.gpsimd.dma_start(out=bk2[:], in_=bk[128:256, :])
        nc.gpsimd.dma_start(out=bq1[:], in_=bq[0:128, :])
        nc.gpsimd.dma_start(out=bq2[:], in_=bq[128:256, :])
        nc.gpsimd.dma_start(out=bvt[:], in_=bv[0:128, :])
        nc.gpsimd.dma_start(out=lam_t[:], in_=lamv[:, :])

        qkv = ctx.enter_context(tc.tile_pool(name="qkv", bufs=1))
        qt1 = qkv.tile([128, NQ], BF16, tag="qt1")
        qt2 = qkv.tile([128, NQ], BF16, tag="qt2")
        kt1 = qkv.tile([128, S], BF16, tag="kt1")
        kt2 = qkv.tile([128, S], BF16, tag="kt2")
        vv = qkv.tile([128, S], BF16, tag="vv")  # [key%128, kt*128+d] = V[key, d]
        # staging for own-half K/V pieces (k1|k2|v), AllGathered to the pair
        stg = [
            qkv.tile([128, 3, 1024], BF16, tag=f"stg{p}", name=f"stg{p}")
            for p in range(2)
        ]

        epool = ctx.enter_context(tc.tile_pool(name="e", bufs=3))
        papool = ctx.enter_context(tc.tile_pool(name="pacc", bufs=3))
        fpool = ctx.enter_context(tc.tile_pool(name="fin", bufs=2))
        vspool = ctx.enter_context(tc.tile_pool(name="vts", bufs=2))

        # ---------------- projections (sequential, quad-chunk jobs) ----
        # The exp stream has ~1 step of lookahead (PSUM-bound), so
        # interleaving proj into attention stalls it 1:1 — strictly
        # ordered proj first is as fast and much simpler. Scoped PSUM
        # pool: banks are reused by the attention pools afterwards.
        from contextlib import ExitStack as _ES

        with _ES() as pctx:
            ppool = pctx.enter_context(
                tc.tile_pool(name="ppsum", bufs=2, space="PSUM")
            )
            tpool = pctx.enter_context(
                tc.tile_pool(name="ptr", bufs=2, space="PSUM")
            )

            def proj_job(dst, w_t, b_t, pair, tag, first=False, vstg=None):
                # dst: pre-sliced [128, 1024] AP for the pair's output, or
                # None with vstg set: V path (transpose into vstg[:, 2, :])
                chunks = (2 * pair, 2 * pair + 1)
                ps = ppool.tile([128, 2, SQC], F32, tag="pp", name=f"ps_{tag}")
                if first:
                    # chunk-outer so compute starts as soon as chunk 0 lands
                    for ci in range(2):
                        for e in range(NE):
                            nc.tensor.matmul(
                                ps[:, ci, :], w_t[:, e, :],
                                xt_all[:, chunks[ci], e, :],
                                start=(e == 0), stop=(e == NE - 1),
                            )
                else:
                    # weight-stationary: one LDW feeds 2 matmuls
                    for e in range(NE):
                        for ci in range(2):
                            nc.tensor.matmul(
                                ps[:, ci, :], w_t[:, e, :],
                                xt_all[:, chunks[ci], e, :],
                                start=(e == 0), stop=(e == NE - 1),
                            )
                if vstg is None:
                    nc.scalar.activation(
                        dst, ps[:], AF.Identity, bias=b_t[:, 0:1]
                    )
                else:
                    vt_s = vspool.tile(
                        [128, 2, SQC], BF16, tag="vts", name=f"vt_{tag}"
                    )
                    nc.scalar.activation(
                        vt_s[:], ps[:], AF.Identity, bias=b_t[:, 0:1]
                    )
                    for ci in range(2):
                        for j in range(SQC // 128):
                            tr = tpool.tile(
                                [128, 128], BF16, tag="vtr",
                                name=f"vtr_{tag}_{ci}_{j}",
                            )
                            nc.tensor.transpose(
                                tr[:], vt_s[:, ci, j * 128 : (j + 1) * 128],
                                ident[:],
                            )
                            col = (ci * (SQC // 128) + j) * 128
                            nc.vector.tensor_copy(
                                vstg[:, 2, col : col + 128], tr[:]
                            )

            def stage_and_gather(p):
                # local order: stage done -> AllGather -> load both slots
                nc.sync.dma_start(
                    out=kv_in[p][:, :],
                    in_=stg[p][:].rearrange("a b c -> a (b c)"),
                )
                nc.gpsimd.collective_compute(
                    "AllGather",
                    mybir.AluOpType.bypass,
                    replica_groups=[[0, 1], [2, 3], [4, 5], [6, 7]],
                    ins=[kv_in[p][:, :].opt()],
                    outs=[kv_out[p][:, :, :].opt()],
                )
                for slot in range(2):
                    base = slot * 2048 + p * 1024
                    nc.sync.dma_start(
                        out=kt1[:, base : base + 1024],
                        in_=kv_out[p][slot, :, 0:1024],
                    )
                    nc.sync.dma_start(
                        out=kt2[:, base : base + 1024],
                        in_=kv_out[p][slot, :, 1024:2048],
                    )
                    nc.sync.dma_start(
                        out=vv[:, base : base + 1024],
                        in_=kv_out[p][slot, :, 2048:3072],
                    )

            # piece p covers own chunks (2p, 2p+1); K/V land in the staging
            # tile (the gather delivers batch-order keys), Q goes direct
            for p in range(2):
                qsl = slice(p * 1024, (p + 1) * 1024)
                proj_job(
                    stg[p][:, 0, :], wk1, bk1, p, f"k1_{p}", first=(p == 0)
                )
                proj_job(stg[p][:, 1, :], wk2, bk2, p, f"k2_{p}")
                proj_job(None, wvt, bvt, p, f"v_{p}", vstg=stg[p])
                stage_and_gather(p)
                proj_job(qt1[:, qsl], wq1, bq1, p, f"q1_{p}")
                proj_job(qt2[:, qsl], wq2, bq2, p, f"q2_{p}")

        # PSUM after proj: spool 2x2 + rpool 2 + opool 2 banks = 8
        spool = ctx.enter_context(tc.tile_pool(name="spsum", bufs=2, space="PSUM"))
        rpool = ctx.enter_context(tc.tile_pool(name="rpsum", bufs=1, space="PSUM"))
        opool = ctx.enter_context(tc.tile_pool(name="opsum", bufs=1, space="PSUM"))

        # ---------------- attention ----------------
        for sup in range(NSUP):
            qa = sup * SUP
            o1 = opool.tile([128, SUP], F32, tag="o1", name=f"o1_{sup}")
            o2 = opool.tile([128, SUP], F32, tag="o2", name=f"o2_{sup}")
            pacc = papool.tile([128, 2, SUP], BF16, tag="pacc", name=f"pa_{sup}")

            # key-tile order follows gather-piece availability:
            # piece A -> kt 0-7 & 16-23, piece B -> kt 8-15 & 24-31
            ktlist = (
                list(range(0, 8)) + list(range(16, 24))
                + list(range(8, 16)) + list(range(24, 32))
            )
            for idx, kt in enumerate(ktlist):
                ksl = slice(kt * 128, (kt + 1) * 128)
                s12 = spool.tile(
                    [128, 2, SUP], F32, tag="s12", name=f"s12_{sup}_{kt}"
                )
                nc.tensor.matmul(
                    s12[:, 0, :], kt1[:, ksl], qt1[:, qa : qa + SUP],
                    start=True, stop=True,
                )
                nc.tensor.matmul(
                    s12[:, 1, :], kt2[:, ksl], qt2[:, qa : qa + SUP],
                    start=True, stop=True,
                )
                e12 = epool.tile(
                    [128, 2, SUP], BF16, tag="e12", name=f"e_{sup}_{kt}"
                )
                nc.scalar.activation(
                    e12[:], s12[:], AF.Exp, scale=float(INV_SQRT_D)
                )
                if idx == 0:
                    nc.vector.tensor_copy(pacc[:], e12[:])
                else:
                    nc.vector.tensor_add(pacc[:], pacc[:], e12[:])
                nc.tensor.matmul(
                    o1[:], vv[:, ksl], e12[:, 0, :],
                    start=(idx == 0), stop=(idx == NSK - 1),
                )
                nc.tensor.matmul(
                    o2[:], vv[:, ksl], e12[:, 1, :],
                    start=(idx == 0), stop=(idx == NSK - 1),
                )

            # ---- finalize this super ----
            rs_b = rpool.tile([128, 2, SUP], F32, tag="rsb", name=f"rsb_{sup}")
            for comp in range(2):
                nc.tensor.matmul(
                    rs_b[:, comp, :], ones_sq[:], pacc[:, comp, :],
                    start=True, stop=True,
                )
            # o evacuations on ScalarE (parallel with DVE's reciprocal)
            o1s = fpool.tile([128, SUP], BF16, tag="o1s", name=f"o1s_{sup}")
            nc.scalar.activation(o1s[:], o1[:], AF.Identity)
            o2s = fpool.tile([128, SUP], BF16, tag="o2s", name=f"o2s_{sup}")
            nc.scalar.activation(o2s[:], o2[:], AF.Identity, scale=lam_t[:, 0:1])

            ir = fpool.tile([128, 2, SUP], F32, tag="ir", name=f"ir_{sup}")
            nc.vector.reciprocal_approx_fast(ir[:], rs_b[:])
            t1 = fpool.tile([128, SUP], F32, tag="t1", name=f"t1_{sup}")
            nc.vector.tensor_mul(t1[:], o1s[:], ir[:, 0, :])
            t2 = fpool.tile([128, SUP], F32, tag="t2", name=f"t2_{sup}")
            nc.vector.tensor_mul(t2[:], o2s[:], ir[:, 1, :])
            o_t = fpool.tile([128, SUP], F32, tag="ot", name=f"ot_{sup}")
            nc.vector.tensor_sub(o_t[:], t1[:], t2[:])
            nc.sync.dma_start(out=out[:, qa : qa + SUP], in_=o_t[:])

    nc.compile()
    return nc


_NC = None


def _get_nc():
    global _NC
    if _NC is None:
        _NC = _build()
    return _NC


def kernel(X, lam, Wq, bq, Wk, bk, Wv, bv):
    import ml_dtypes

    BF = ml_dtypes.bfloat16
    X = np.asarray(X, dtype=np.float32)
    lam_f = float(np.asarray(lam))
    Wq_b = np.ascontiguousarray(np.asarray(Wq, np.float32).astype(BF))
    Wk_b = np.ascontiguousarray(np.asarray(Wk, np.float32).astype(BF))
    Wv_b = np.ascontiguousarray(np.asarray(Wv, np.float32).astype(BF))
    bq_c = np.asarray(bq, np.float32).reshape(2 * D, 1).copy()
    bk_c = np.asarray(bk, np.float32).reshape(2 * D, 1).copy()
    bv_c = np.asarray(bv, np.float32).reshape(D, 1).copy()
    lam_v = np.full((128, 1), lam_f, np.float32)

    nc = _get_nc()

    in_maps = []
    for core in range(8):
        b, h = divmod(core, 2)
        xb = X[b]
        if h == 0:
            xr = xb
        else:
            xr = np.concatenate([xb[NQ:], xb[:NQ]], axis=0)
        xt_a = np.ascontiguousarray(xr.T.astype(BF))
        in_maps.append(
            {
                "xt": xt_a,
                "wq": Wq_b,
                "wk": Wk_b,
                "wv": Wv_b,
                "bq": bq_c,
                "bk": bk_c,
                "bv": bv_c,
                "lamv": lam_v,
            }
        )

    global LAST_RESULT
    kwargs = {}
    if TRACE:
        import tempfile

        tdir = tempfile.mkdtemp(dir=TRACE_DIR) if TRACE_DIR else None
        kwargs = dict(trace=True, tmpdir=tdir)
    res = run_bass_kernel_spmd(nc, in_maps, list(range(8)), **kwargs)
    LAST_RESULT = res

    o = np.empty((B, S, D), np.float32)
    for core in range(8):
        b, h = divmod(core, 2)
        o[b, h * NQ : (h + 1) * NQ, :] = np.asarray(
            res.results[core]["o"], np.float32
        ).T
    return o
